# revision 4
# baseline (speedup 1.0000x reference)
"""Sparse attention (RoPE'd Q=K, strictly-causal unnormalized scores @ V).

  Q: (1, 4, 2048, 8192) f32   V: (1, 1, 2048, 256) f32
  out = tril(QR @ QR^T, -1) @ V   per head, V broadcast over heads.

Sharding: 8 cores = 4 heads x 2 halves of the N=8192 contraction dim.
The causal mask is elementwise, so masked-scores @ V is additive over
N-slices: each core computes a full (2048, 256) partial output from its
(2048, 4096) slice of QR; host sums the two halves per head.

Device kernel (SPMD, per core):
  - QR^T slice [4096, 2048] resident in SBUF as bf16 (32 tiles of [128, 2048])
  - scores tile (a, B) = QRT[:, aP:(a+1)P].T @ QRT[:, B*512:(B+1)*512]
    accumulated over 32 k-tiles in PSUM (f32); only tiles on/above the
    diagonal of the [s, t] layout are computed (scores is symmetric, and
    the strict-lower mask in [t, s] layout is the strict-upper in [s, t],
    which is exactly what the second matmul wants as lhsT - no transpose).
  - diagonal-crossing tiles multiplied by a staircase mask during the
    PSUM -> SBUF copy; scores kept f32 in SBUF.
  - out t-tile b: sum_{a<=b} scores[a, b-slice].T @ V[a] as float32r
    matmuls (full PE rate at free dim 256), PSUM f32 -> SBUF -> DRAM.

RoPE, transpose, bf16 cast and the final half-sum happen on host.
"""

import math

import numpy as np

THETA = 2.0**16
TWO_PI = 2.0 * math.pi

B, NH, T, N, D = 1, 4, 2048, 8192, 256
NSPLIT = 2
NCORES = NH * NSPLIT
NC_FEAT = N // NSPLIT  # 4096 features per core
P = 128
KT = NC_FEAT // P  # 32 contraction tiles
TT = T // P  # 16 row/col tiles of the scores matrix
BW = 512  # scores column-block width
NB = T // BW  # 4 column blocks
DIAG = BW // P  # 4 mask variants

_COMPILED = None


def _rope_host(Q):
    """Match reference: f32 phases, mod 1, cos/sin, interleaved rotate."""
    idx = (np.floor(np.arange(N, dtype=np.float32) / 2.0) * 2.0).astype(np.float32)
    freqs = (1.0 / (THETA ** (idx / np.float32(N))) / np.float32(TWO_PI)).astype(
        np.float32
    )
    t = np.arange(T, dtype=np.float32)
    phases = t[:, None] * freqs[None, :]
    ang = np.float32(TWO_PI) * (phases % np.float32(1.0))
    cos = np.cos(ang).astype(np.float32)
    sin = np.sin(ang).astype(np.float32)
    Qr = np.empty_like(Q)
    Qr[..., 0::2] = -Q[..., 1::2]
    Qr[..., 1::2] = Q[..., 0::2]
    return Q * cos + Qr * sin


def _masks_host():
    """mask[i][si, tj] = 1 if si + 128*i < tj, for the 4 diagonal variants."""
    si = np.arange(P)[:, None]
    tj = np.arange(BW)[None, :]
    return np.concatenate(
        [(si + P * i < tj).astype(np.float32) for i in range(DIAG)], axis=0
    )  # [512, 512]


def _build():
    import concourse.tile as tile
    from concourse import bacc, mybir

    nc = bacc.Bacc(
        "TRN2",
        target_bir_lowering=False,
        debug=False,
        enable_asserts=False,
        num_devices=NCORES,
    )
    qrt = nc.dram_tensor("qrt", [NC_FEAT, T], mybir.dt.bfloat16, kind="ExternalInput").ap()
    v = nc.dram_tensor("v", [T, D], mybir.dt.float32, kind="ExternalInput").ap()
    masks = nc.dram_tensor(
        "masks", [DIAG * P, BW], mybir.dt.float32, kind="ExternalInput"
    ).ap()
    out = nc.dram_tensor("out", [T, D], mybir.dt.float32, kind="ExternalOutput").ap()

    f32 = mybir.dt.float32
    f32r = mybir.dt.float32r
    bf16 = mybir.dt.bfloat16

    with tile.TileContext(nc) as tc:
        with (
            tc.tile_pool(name="qrt", bufs=KT) as qp,
            tc.tile_pool(name="vp", bufs=TT) as vp,
            tc.tile_pool(name="mk", bufs=DIAG) as mp,
            tc.tile_pool(name="sc", bufs=TT + 2) as sp,
            tc.tile_pool(name="ob", bufs=3) as op_,
            tc.tile_pool(name="ps", bufs=3, space="PSUM") as pp,
            tc.tile_pool(name="po", bufs=2, space="PSUM") as pop,
        ):
            qtiles = []
            for k in range(KT):
                qt = qp.tile([P, T], bf16)
                nc.sync.dma_start(out=qt, in_=qrt[k * P : (k + 1) * P, :])
                qtiles.append(qt)
            vtiles = []
            for a in range(TT):
                vt32 = vp.tile([P, D], f32, tag="v32")
                nc.sync.dma_start(out=vt32, in_=v[a * P : (a + 1) * P, :])
                vt = vp.tile([P, D], f32r, tag="vr")
                nc.vector.tensor_copy(vt, vt32)
                vtiles.append(vt)
            mtiles = []
            for i in range(DIAG):
                mt = mp.tile([P, BW], f32)
                nc.sync.dma_start(out=mt, in_=masks[i * P : (i + 1) * P, :])
                mtiles.append(mt)

            for Bb in range(NB):
                stiles = []
                for a in range(DIAG * Bb + DIAG):
                    ps = pp.tile([P, BW], f32)
                    for k in range(KT):
                        nc.tensor.matmul(
                            ps,
                            lhsT=qtiles[k][:, a * P : (a + 1) * P],
                            rhs=qtiles[k][:, Bb * BW : (Bb + 1) * BW],
                            start=(k == 0),
                            stop=(k == KT - 1),
                        )
                    st = sp.tile([P, BW], f32r)
                    i = a - DIAG * Bb
                    if i >= 0:
                        nc.vector.tensor_mul(st, ps, mtiles[i])
                    else:
                        nc.vector.tensor_copy(st, ps)
                    stiles.append(st)
                for j in range(DIAG):
                    b = DIAG * Bb + j
                    po = pop.tile([P, D], f32)
                    for a in range(b + 1):
                        nc.tensor.matmul(
                            po,
                            lhsT=stiles[a][:, j * P : (j + 1) * P],
                            rhs=vtiles[a],
                            start=(a == 0),
                            stop=(a == b),
                        )
                    ot = op_.tile([P, D], f32)
                    nc.vector.tensor_copy(ot, po)
                    nc.sync.dma_start(out=out[b * P : (b + 1) * P, :], in_=ot)

    nc.compile()
    return nc


def _get_compiled():
    global _COMPILED
    if _COMPILED is None:
        _COMPILED = _build()
    return _COMPILED


def kernel(Q, V, _want_results=False, **_unused):
    import ml_dtypes

    from concourse import bass_utils

    Q = np.asarray(Q, dtype=np.float32)
    V = np.asarray(V, dtype=np.float32)

    QR = _rope_host(Q)  # (1, 4, 2048, 8192) f32
    masks_np = _masks_host()
    v_np = np.ascontiguousarray(V[0, 0])  # (2048, 256) f32

    in_maps = []
    for h in range(NH):
        for half in range(NSPLIT):
            sl = QR[0, h, :, half * NC_FEAT : (half + 1) * NC_FEAT]
            qrt_c = np.ascontiguousarray(sl.T).astype(ml_dtypes.bfloat16)
            in_maps.append({"qrt": qrt_c, "v": v_np, "masks": masks_np})

    nc = _get_compiled()
    res = bass_utils.run_bass_kernel_spmd(nc, in_maps, core_ids=list(range(NCORES)))

    out = np.empty((B, NH, T, D), dtype=np.float32)
    for h in range(NH):
        out[0, h] = res.results[2 * h]["out"] + res.results[2 * h + 1]["out"]
    if _want_results:
        return out, res
    return out


if __name__ == "__main__":
    rng = np.random.default_rng(0)
    Q = (rng.standard_normal((B, NH, T, N)) * 0.02).astype(np.float32)
    V = rng.standard_normal((B, 1, T, D)).astype(np.float32)
    out = kernel(Q=Q, V=V)
    print("out", out.shape, out.dtype, float(np.abs(out).max()))
